# revision 10
# baseline (speedup 1.0000x reference)
"""GAT (2-layer GATConv + FF head) on 8 Trainium2 NeuronCores.

Strategy (per sharding hint): nodes + incident edges partitioned by
destination across 8 cores; per-edge softmax/scatter local to the
destination shard via one-hot matmul-scatter into PSUM; small weights
replicated.

v2 — minimal host->device footprint (~2.4MB/core vs 17.7MB in v1):
  - x is uploaded SHARDED ([128, N/8] fp16 slice of x^T per core) and
    AllGathered on device; the layer-1 dense is then computed replicated.
  - One combined gather table per layer: row = [h(256) | a_src(4) |
    a_dst(4) | pad] in fp16, 384 cols (768B rows — SWDGE needs 256B
    multiples), so each edge needs ONE wide gather (h + a_src) plus one
    narrow 256B gather for a_dst of the destination (local-row table
    written directly by small dense matmuls — no gather-based own-narrow
    build, no node-major fixup copies).
  - Gather indices uploaded compact ([16, n] int16, broadcast to the
    SWDGE-wrapped [128, n] layout on device); per-edge dst-slot table
    uploaded int8; iota/identity/bias-broadcast built on device.
  - Layer-2 exchange is a single AllGather of the combined table.

Message path fp16; accumulation fp32 PSUM; per-edge logits fp32.
"""
import sys
sys.path.insert(0, "/opt/trn_rl_repo")

import numpy as np
from contextlib import ExitStack

import concourse.bass as bass
import concourse.bacc as bacc
import concourse.tile as tile
import concourse.mybir as mybir
from concourse.bass_utils import run_bass_kernel_spmd

dt = mybir.dt
OP = mybir.AluOpType
ACT = mybir.ActivationFunctionType

NCORES = 8
H = 4
NEG_SLOPE = 0.2
TW = 384                  # table row width (fp16) -> 768B, multiple of 256B


# ----------------------------------------------------------------------------
# host-side prep
# ----------------------------------------------------------------------------

def _wrap16(idx):
    """Compact SWDGE index layout [16, n/16]: index i -> partition i%16,
    free offset i//16 (device broadcasts to the 8 replica groups)."""
    n = len(idx)
    assert n % 128 == 0
    return np.ascontiguousarray(np.asarray(idx, np.int16).reshape(n // 16, 16).T)


class Sched:
    """Static, core-uniform per-tile chunk schedule."""

    def __init__(self, n_lo, n_hi):
        self.n_lo = n_lo          # [NT] chunks for lo-half gathers
        self.n_hi = n_hi          # [NT] chunks for hi-half gathers
        self.ct = [a + b for a, b in zip(n_lo, n_hi)]
        self.base = np.concatenate([[0], np.cumsum(self.ct)]).astype(int)
        self.total = int(self.base[-1])  # total chunks per core


def _prep(x, edge_index, W1, att_src1, att_dst1, b1, W2, att_src2, att_dst2,
          b2, ff1_w, ff1_b, ff2_w, ff2_b):
    N, IN = x.shape
    F = W1.shape[1]               # 256
    C1 = F // H
    C2 = W2.shape[1] // H
    NSH = N // NCORES
    NT = (NSH + 127) // 128
    SPLIT = (N // 2) // 128 * 128

    ar = np.arange(N, dtype=np.int64)
    src = np.concatenate([edge_index[0], ar])
    dst = np.concatenate([edge_index[1], ar])

    shard = dst // NSH
    dstloc_all = dst - shard * NSH

    # group edges per (core, tile, half)
    per = [[[None, None] for _ in range(NT)] for _ in range(NCORES)]
    for k in range(NCORES):
        m = shard == k
        s_k, dl_k = src[m], dstloc_all[m]
        t_k = dl_k // 128
        for t in range(NT):
            mt = t_k == t
            s_t, dl_t = s_k[mt], dl_k[mt]
            lo = s_t < SPLIT
            per[k][t][0] = (s_t[lo], dl_t[lo])
            per[k][t][1] = (s_t[~lo] - SPLIT, dl_t[~lo])

    n_lo = [max((len(per[k][t][0][0]) + 127) // 128 for k in range(NCORES))
            for t in range(NT)]
    n_hi = [max((len(per[k][t][1][0]) + 127) // 128 for k in range(NCORES))
            for t in range(NT)]
    sched = Sched(n_lo, n_hi)

    # per-core edge arrays in schedule order
    idxg_l, idxnd_l, dl8_l = [], [], []
    for k in range(NCORES):
        ig = np.zeros((16, sched.total * 8), np.int16)
        nd = np.zeros((16, sched.total * 8), np.int16)
        d8 = np.full((128, sched.total), -1, np.int8)
        for t in range(NT):
            off = sched.base[t]
            for half, nch in ((0, n_lo[t]), (1, n_hi[t])):
                if nch == 0:
                    continue
                s_t, dl_t = per[k][t][half]
                ne = nch * 128
                sp = np.zeros(ne, np.int64)
                sp[:len(s_t)] = s_t
                ndp = np.zeros(ne, np.int64)
                ndp[:len(dl_t)] = dl_t
                d8p = np.full(ne, -1, np.int8)
                d8p[:len(dl_t)] = (dl_t - t * 128).astype(np.int8)
                ig[:, off * 8:(off + nch) * 8] = _wrap16(sp)
                nd[:, off * 8:(off + nch) * 8] = _wrap16(ndp)
                d8[:, off:off + nch] = d8p.reshape(nch, 128).T
                off += nch
        idxg_l.append(ig)
        idxnd_l.append(nd)
        dl8_l.append(d8)

    # weights (attention vectors folded in as extra output columns)
    def aug(W, a_s, a_d, C):
        v_s = np.einsum("fhc,hc->fh", W.reshape(-1, H, C), a_s)
        v_d = np.einsum("fhc,hc->fh", W.reshape(-1, H, C), a_d)
        return np.concatenate([W, v_s, v_d], axis=1).astype(np.float16)

    W1aug = aug(W1, att_src1, att_dst1, C1)              # [IN, F+8]
    W2aug = aug(W2, att_src2, att_dst2, C2)              # [F, F+8]
    W2aug_pk = np.ascontiguousarray(
        W2aug.reshape(2, 128, F + 8).transpose(1, 0, 2))

    xT16 = np.ascontiguousarray(x.T).astype(np.float16)  # [IN, N]

    const = {
        "w1a": W1aug, "w2a": W2aug_pk,
        "b1row": b1.astype(np.float32).reshape(1, F),
        "b2row": b2.astype(np.float32).reshape(1, C2),
        "f1brow": ff1_b.astype(np.float32).reshape(1, -1),
        "f2brow": ff2_b.astype(np.float32).reshape(1, 2),
        "ff1w16": ff1_w.astype(np.float16),
        "ff2w16": ff2_w.astype(np.float16),
    }

    in_maps = []
    for k in range(NCORES):
        m = dict(const)
        m.update({
            "x_sh": np.ascontiguousarray(xT16[:, k * NSH:(k + 1) * NSH]),
            "idxg": idxg_l[k], "idxnd": idxnd_l[k], "dl8": dl8_l[k],
        })
        in_maps.append(m)

    dims = dict(N=N, IN=IN, F=F, NSH=NSH, NT=NT, SPLIT=SPLIT, C2=C2,
                FH=ff1_w.shape[1])
    return in_maps, sched, dims


# ----------------------------------------------------------------------------
# device program
# ----------------------------------------------------------------------------

def _gather_split(nc, out_ap_fn, tab, idx_sb, n_chunks, elem, q0):
    """Emit dma_gather calls capped at 8 chunks (1024 idxs) each."""
    c0 = 0
    q = q0
    while c0 < n_chunks:
        c1 = min(c0 + 8, n_chunks)
        nc.gpsimd.dma_gather(
            out_ap_fn(c0, c1), tab, idx_sb[:, c0 * 8:c1 * 8],
            num_idxs=(c1 - c0) * 128, num_idxs_reg=(c1 - c0) * 128,
            elem_size=elem, queue_num=q % 4)
        q += 1
        c0 = c1


def _wr_rows(nc, dst, r0, rows, st, width, col0=0):
    """DMA staging [128, G, width] (row r = g*128+p at [p, g]) to DRAM rows
    dst[r0:r0+rows]."""
    g_full = rows // 128
    if g_full:
        nc.sync.dma_start(
            dst[r0:r0 + g_full * 128, :].rearrange("(g p) c -> p g c", p=128),
            st[:, col0:col0 + g_full, :])
    rem = rows - g_full * 128
    if rem:
        nc.sync.dma_start(dst[r0 + g_full * 128:r0 + rows, :],
                          st[0:rem, col0 + g_full, :])


def _build(sched, dims):
    import os, hashlib
    PH = int(os.environ.get('K_PHASES', '5'))
    N, IN, F, NSH, NT = dims["N"], dims["IN"], dims["F"], dims["NSH"], dims["NT"]
    SPLIT, C2, FH = dims["SPLIT"], dims["C2"], dims["FH"]
    FA = F + 8
    NTG = (N + 127) // 128

    nc = bacc.Bacc("TRN2", target_bir_lowering=False, num_devices=NCORES,
                   num_swdge_queues=4)
    # The neuronx compile cache keys on the jit signature only, so a
    # content-named dummy input de-aliases different programs.
    with open(__file__, "rb") as _f:
        _salt = hashlib.sha256(
            _f.read() + repr((sched.n_lo, sched.n_hi, sorted(dims.items()),
                              PH)).encode()).hexdigest()[:16]
    nc.dram_tensor(f"salt_{_salt}", [1, 4], dt.float32, kind="ExternalInput")
    dims["salt_name"] = f"salt_{_salt}"

    # inputs
    x_sh = nc.dram_tensor("x_sh", [IN, NSH], dt.float16, kind="ExternalInput")
    w1a_d = nc.dram_tensor("w1a", [IN, FA], dt.float16, kind="ExternalInput")
    w2a_d = nc.dram_tensor("w2a", [128, 2, FA], dt.float16, kind="ExternalInput")
    b1r_d = nc.dram_tensor("b1row", [1, F], dt.float32, kind="ExternalInput")
    b2r_d = nc.dram_tensor("b2row", [1, C2], dt.float32, kind="ExternalInput")
    f1br_d = nc.dram_tensor("f1brow", [1, FH], dt.float32, kind="ExternalInput")
    f2br_d = nc.dram_tensor("f2brow", [1, 2], dt.float32, kind="ExternalInput")
    ff1w_d = nc.dram_tensor("ff1w16", [C2, FH], dt.float16, kind="ExternalInput")
    ff2w_d = nc.dram_tensor("ff2w16", [FH, 2], dt.float16, kind="ExternalInput")
    idxg_d = nc.dram_tensor("idxg", [16, sched.total * 8], dt.int16,
                            kind="ExternalInput")
    idxnd_d = nc.dram_tensor("idxnd", [16, sched.total * 8], dt.int16,
                             kind="ExternalInput")
    dl8_d = nc.dram_tensor("dl8", [128, sched.total], dt.int8,
                           kind="ExternalInput")

    out_d = nc.dram_tensor("out", [NSH, 2], dt.float32, kind="ExternalOutput")

    with tile.TileContext(nc) as tc, ExitStack() as octx:
        dram = octx.enter_context(tc.tile_pool(name="dram", bufs=1, space="DRAM"))
        cpool = octx.enter_context(tc.tile_pool(name="const", bufs=1))
        stash = octx.enter_context(tc.tile_pool(name="stash", bufs=1))

        # DRAM tables
        x_own = dram.tile([IN, NSH], dt.float16)
        x_all = dram.tile([NCORES * IN, NSH], dt.float16, addr_space="Shared")
        T1 = dram.tile([NTG * 128, TW], dt.float16)
        T2_own = dram.tile([NSH, TW], dt.float16)
        T2_all = dram.tile([N, TW], dt.float16, addr_space="Shared")
        nd1 = dram.tile([NT * 128, 128], dt.float16)
        nd2 = dram.tile([NT * 128, 128], dt.float16)

        # constants in SBUF
        w1a_sb = cpool.tile([IN, FA], dt.float16)
        nc.sync.dma_start(w1a_sb[:], w1a_d[:])
        w2a_sb = cpool.tile([128, 2, FA], dt.float16)
        nc.sync.dma_start(w2a_sb[:], w2a_d[:])
        ff1_sb = cpool.tile([C2, FH], dt.float16)
        nc.sync.dma_start(ff1_sb[:], ff1w_d[:])
        ff2_sb = cpool.tile([FH, 2], dt.float16)
        nc.sync.dma_start(ff2_sb[:], ff2w_d[:])
        xme_sb = cpool.tile([IN, NSH], dt.float16)
        nc.sync.dma_start(xme_sb[:], x_sh[:])

        # iota / identity generated on device
        ii16 = cpool.tile([128, 128], dt.int16)
        nc.gpsimd.iota(ii16[:], pattern=[[1, 128]], base=0, channel_multiplier=0)
        iota16 = cpool.tile([128, 128], dt.float16)
        nc.vector.tensor_copy(iota16[:], ii16[:])
        ip16 = cpool.tile([128, 1], dt.int16)
        nc.gpsimd.iota(ip16[:], pattern=[[0, 1]], base=0, channel_multiplier=1)
        ipf = cpool.tile([128, 1], dt.float16)
        nc.vector.tensor_copy(ipf[:], ip16[:])
        ident16 = cpool.tile([128, 128], dt.float16)
        nc.vector.tensor_tensor(ident16[:], iota16[:],
                                ipf[:].broadcast_to([128, 128]),
                                op=OP.is_equal)

        # bias rows broadcast to 128 partitions via ones-matmul
        ones1 = cpool.tile([1, 128], dt.float32)
        nc.vector.memset(ones1[:], 1.0)
        b1_sb = cpool.tile([128, F], dt.float32)
        b2_sb = cpool.tile([128, C2], dt.float32)
        f1b_sb = cpool.tile([128, FH], dt.float32)
        f2b_sb = cpool.tile([128, 2], dt.float32)
        with ExitStack() as bctx:
            bp = bctx.enter_context(tc.tile_pool(name="bp", bufs=1))
            bpp = bctx.enter_context(tc.tile_pool(name="bpp", bufs=2, space="PSUM"))
            for (row_d, w, sb) in ((b1r_d, F, b1_sb), (b2r_d, C2, b2_sb),
                                   (f1br_d, FH, f1b_sb), (f2br_d, 2, f2b_sb)):
                rsb = bp.tile([1, w], dt.float32, tag="rsb")
                nc.sync.dma_start(rsb[:], row_d[:])
                psb = bpp.tile([128, w], dt.float32, tag="psb")
                nc.tensor.matmul(psb[:], ones1[:], rsb[:], start=True, stop=True)
                nc.vector.tensor_copy(sb[:], psb[:])

        # gather indices: broadcast compact [16, n] to wrapped [128, n]
        idxg_sb = cpool.tile([128, sched.total * 8], dt.int16)
        idxnd_sb = cpool.tile([128, sched.total * 8], dt.int16)
        for g in range(8):
            nc.sync.dma_start(idxg_sb[g * 16:(g + 1) * 16, :], idxg_d[:])
            nc.sync.dma_start(idxnd_sb[g * 16:(g + 1) * 16, :], idxnd_d[:])
        dl8_sb = cpool.tile([128, sched.total], dt.int8)
        nc.sync.dma_start(dl8_sb[:], dl8_d[:])
        dl16 = cpool.tile([128, sched.total], dt.float16)
        nc.vector.tensor_copy(dl16[:], dl8_sb[:])

        # layer-1 hidden transposed, kept in SBUF for the layer-2 dense
        h1T = stash.tile([128, 2, NT, 128], dt.float16)
        out_stage = stash.tile([128, NT, 2], dt.float32)

        # ------------------------------------------------------------------
        # phase A: AllGather x; replicated layer-1 dense -> T1; own-narrow
        # dense from x_sh -> nd1
        # ------------------------------------------------------------------
        nc.sync.dma_start(x_own[:], x_sh[:])
        nc.gpsimd.collective_compute(
            "AllGather", OP.bypass, replica_groups=[list(range(NCORES))],
            ins=[x_own[:].opt()], outs=[x_all[:].opt()])

        with ExitStack() as ctx:
            xp = ctx.enter_context(tc.tile_pool(name="xp", bufs=2))
            pp = ctx.enter_context(tc.tile_pool(name="pp", bufs=4, space="PSUM"))
            sp = ctx.enter_context(tc.tile_pool(name="sp", bufs=2))

            # own-narrow dense: nd1 rows t*128+p = [a_src | a_dst] of own node
            G = 8
            t0 = 0
            while t0 < NT:
                g = min(G, NT - t0)
                nst = sp.tile([128, G, 8], dt.float16, tag="nst")
                for j in range(g):
                    t = t0 + j
                    rows = min(128, NSH - t * 128)
                    psn = pp.tile([128, 8], dt.float32, tag="psn")
                    nc.tensor.matmul(psn[0:rows, :],
                                     xme_sb[:, t * 128:t * 128 + rows],
                                     w1a_sb[:, F:FA], start=True, stop=True)
                    nc.scalar.activation(nst[0:rows, j, :], psn[0:rows, :],
                                         ACT.Copy)
                _wr_rows(nc, nd1[:, 0:8], t0 * 128,
                         min(g * 128, NT * 128 - t0 * 128), nst, 8)
                t0 += g

            # replicated full dense -> T1 (shard-major == node-major rows)
            for k in range(NCORES):
                xk = xp.tile([IN, NSH], dt.float16, tag="xk")
                nc.sync.dma_start(xk[:], x_all[k * IN:(k + 1) * IN, :])
                t0 = 0
                while t0 < NT:
                    g = min(G, NT - t0)
                    hst = sp.tile([128, G, FA], dt.float16, tag="hst")
                    for j in range(g):
                        t = t0 + j
                        rows = min(128, NSH - t * 128)
                        ps = pp.tile([128, FA], dt.float32, tag="ps")
                        nc.tensor.matmul(ps[0:rows, :],
                                         xk[:, t * 128:t * 128 + rows],
                                         w1a_sb[:], start=True, stop=True)
                        nc.scalar.activation(hst[0:rows, j, :], ps[0:rows, :],
                                             ACT.Copy)
                    rows_t = min(g * 128, NSH - t0 * 128)
                    _wr_rows(nc, T1[:, 0:FA], k * NSH + t0 * 128, rows_t, hst, FA)
                    t0 += g

        # ------------------------------------------------------------------
        # edge phase (shared for both layers)
        # ------------------------------------------------------------------
        def edge_phase(ctx, name, tab_lo, tab_hi, nd_tab, evict):
            ep = ctx.enter_context(tc.tile_pool(name=name + "e", bufs=2))
            pp = ctx.enter_context(tc.tile_pool(name=name + "p", bufs=2,
                                                space="PSUM"))
            for t in range(NT):
                ct = sched.ct[t]
                if ct == 0:
                    continue
                nlo, nhi = sched.n_lo[t], sched.n_hi[t]
                b0 = sched.base[t]

                g = ep.tile([128, ct, TW], dt.float16, tag="g")
                ndt = ep.tile([128, ct, 128], dt.float16, tag="ndt")
                if nlo:
                    _gather_split(nc, lambda a, b: g[:, a:b, :], tab_lo,
                                  idxg_sb[:, b0 * 8:(b0 + nlo) * 8], nlo, TW, 0)
                if nhi:
                    _gather_split(
                        nc, lambda a, b: g[:, nlo + a:nlo + b, :], tab_hi,
                        idxg_sb[:, (b0 + nlo) * 8:(b0 + ct) * 8], nhi, TW, 2)
                _gather_split(nc, lambda a, b: ndt[:, a:b, :], nd_tab,
                              idxnd_sb[:, b0 * 8:(b0 + ct) * 8], ct, 128, 1)

                # alpha = lrelu(a_src[src] + a_dst[dst]); ex = exp(alpha)
                alpha = ep.tile([128, ct, H], dt.float32, tag="alpha")
                nc.vector.tensor_tensor(alpha[:], g[:, :, F:F + H],
                                        ndt[:, :, H:2 * H], op=OP.add)
                nc.vector.scalar_tensor_tensor(
                    alpha[:], alpha[:], float(NEG_SLOPE), alpha[:],
                    op0=OP.mult, op1=OP.max)
                rhs = ep.tile([128, ct, F + H], dt.float16, tag="rhs")
                nc.scalar.activation(rhs[:, :, F:F + H], alpha[:], ACT.Exp)
                # fold: rhs[:, :, 0:F] = h * ex (per-head broadcast)
                nc.vector.tensor_tensor(
                    rhs[:, :, 0:F].rearrange("p c (h d) -> p c h d", h=H),
                    g[:, :, 0:F].rearrange("p c (h d) -> p c h d", h=H),
                    rhs[:, :, F:F + H].unsqueeze(3).broadcast_to(
                        [128, ct, H, F // H]),
                    op=OP.mult)
                # one-hot + matmul-scatter
                oh = ep.tile([128, ct, 128], dt.float16, tag="oh")
                nc.vector.tensor_tensor(
                    oh[:],
                    iota16[:].unsqueeze(1).broadcast_to([128, ct, 128]),
                    dl16[:, b0:b0 + ct].unsqueeze(2).broadcast_to(
                        [128, ct, 128]),
                    op=OP.is_equal)
                ps = pp.tile([128, F + H], dt.float32, tag="ps")
                for c in range(ct):
                    nc.tensor.matmul(ps[:], oh[:, c, :], rhs[:, c, :],
                                     start=(c == 0), stop=(c == ct - 1))
                evict(ep, pp, t, ps)

        # ---- layer 1 evict: h1 = relu(agg/den + b1); h1T stash ----
        def evict1(ep, pp, t, ps):
            rcp = ep.tile([128, H], dt.float32, tag="rcp")
            nc.vector.reciprocal(rcp[:], ps[:, F:F + H])
            pre = ep.tile([128, F], dt.float32, tag="pre")
            nc.vector.tensor_tensor(
                pre[:].rearrange("p (h d) -> p h d", h=H),
                ps[:, 0:F].rearrange("p (h d) -> p h d", h=H),
                rcp[:].unsqueeze(2).broadcast_to([128, H, F // H]), op=OP.mult)
            nc.vector.tensor_tensor(pre[:], pre[:], b1_sb[:], op=OP.add)
            h1r = ep.tile([128, F], dt.float16, tag="h1r")
            nc.scalar.activation(h1r[:], pre[:], ACT.Relu)
            for b in range(2):
                tp = pp.tile([128, 128], dt.float16, tag="tp")
                nc.tensor.transpose(tp[:], h1r[:, b * 128:(b + 1) * 128],
                                    ident16[:])
                nc.scalar.activation(h1T[:, b, t, :], tp[:], ACT.Copy)

        if PH >= 2:
            with ExitStack() as ctx:
                edge_phase(ctx, "l1", T1[0:SPLIT, :], T1[SPLIT:NTG * 128, :],
                           nd1[:], evict1)

        # ------------------------------------------------------------------
        # phase C: layer-2 dense on own rows -> T2_own + nd2
        # ------------------------------------------------------------------
        if PH >= 3:
            with ExitStack() as ctx:
                cp = ctx.enter_context(tc.tile_pool(name="cp", bufs=2))
                pp = ctx.enter_context(tc.tile_pool(name="cpp", bufs=4,
                                                    space="PSUM"))
                G = 8
                t0 = 0
                while t0 < NT:
                    g = min(G, NT - t0)
                    hst = cp.tile([128, G, FA], dt.float16, tag="hst")
                    for j in range(g):
                        t = t0 + j
                        ps = pp.tile([128, FA], dt.float32, tag="ps")
                        for b in range(2):
                            nc.tensor.matmul(ps[:], h1T[:, b, t, :],
                                             w2a_sb[:, b, :],
                                             start=(b == 0), stop=(b == 1))
                        nc.scalar.activation(hst[:, j, :], ps[:], ACT.Copy)
                    rows_t = min(g * 128, NSH - t0 * 128)
                    _wr_rows(nc, T2_own[:, 0:FA], t0 * 128, rows_t, hst, FA)
                    _wr_rows(nc, nd2[:, 0:8], t0 * 128,
                             min(g * 128, NT * 128 - t0 * 128),
                             hst[:, :, F:FA], 8)
                    t0 += g

        # ------------------------------------------------------------------
        # phase D: exchange (single AllGather, node-major result)
        # ------------------------------------------------------------------
        if PH >= 4:
            nc.gpsimd.collective_compute(
                "AllGather", OP.bypass, replica_groups=[list(range(NCORES))],
                ins=[T2_own[:].opt()], outs=[T2_all[:].opt()])

        # ---- layer 2 evict: h2 = relu(mean_h(agg/den) + b2); FF head ----
        def evict2(ep, pp, t, ps):
            rcp = ep.tile([128, H], dt.float32, tag="rcp")
            nc.vector.reciprocal(rcp[:], ps[:, F:F + H])
            pre = ep.tile([128, H, C2], dt.float32, tag="pre")
            nc.vector.tensor_tensor(
                pre[:], ps[:, 0:F].rearrange("p (h d) -> p h d", h=H),
                rcp[:].unsqueeze(2).broadcast_to([128, H, C2]), op=OP.mult)
            red = ep.tile([128, C2], dt.float32, tag="red")
            nc.vector.tensor_reduce(red[:], pre[:].transpose([0, 2, 1]),
                                    axis=mybir.AxisListType.X, op=OP.add)
            nc.vector.scalar_tensor_tensor(red[:], red[:], 1.0 / H, b2_sb[:],
                                           op0=OP.mult, op1=OP.add)
            h2 = ep.tile([128, 128], dt.float16, tag="h2")
            nc.vector.memset(h2[:, C2:128], 0.0)
            nc.scalar.activation(h2[:, 0:C2], red[:], ACT.Relu)
            # FF: out = relu(h2 @ ff1 + b1f) @ ff2 + b2f  (square transposes)
            tp = pp.tile([128, 128], dt.float16, tag="tp2", bufs=1)
            nc.tensor.transpose(tp[:], h2[:], ident16[:])
            h2T = ep.tile([C2, 128], dt.float16, tag="h2T")
            nc.scalar.activation(h2T[:], tp[0:C2, :], ACT.Copy)
            pf1 = pp.tile([128, FH], dt.float32, tag="pf1", bufs=1)
            nc.tensor.matmul(pf1[:], h2T[:], ff1_sb[:], start=True, stop=True)
            f1p = ep.tile([128, FH], dt.float32, tag="f1p")
            nc.vector.tensor_tensor(f1p[:], pf1[:], f1b_sb[:], op=OP.add)
            f1 = ep.tile([128, 128], dt.float16, tag="f1")
            nc.vector.memset(f1[:, FH:128], 0.0)
            nc.scalar.activation(f1[:, 0:FH], f1p[:], ACT.Relu)
            tpf = pp.tile([128, 128], dt.float16, tag="tpf", bufs=1)
            nc.tensor.transpose(tpf[:], f1[:], ident16[:])
            f1T = ep.tile([FH, 128], dt.float16, tag="f1T")
            nc.scalar.activation(f1T[:], tpf[0:FH, :], ACT.Copy)
            pf2 = pp.tile([128, 2], dt.float32, tag="pf2", bufs=1)
            nc.tensor.matmul(pf2[:], f1T[:], ff2_sb[:], start=True, stop=True)
            nc.vector.tensor_tensor(out_stage[:, t, :], pf2[:], f2b_sb[:],
                                    op=OP.add)

        if PH >= 5:
            with ExitStack() as ctx:
                edge_phase(ctx, "l2", T2_all[0:SPLIT, :], T2_all[SPLIT:N, :],
                           nd2[:], evict2)

        # final output
        if PH < 5:
            nc.vector.memset(out_stage[:], 0.0)
        full = (NSH // 128) * 128
        if full:
            nc.sync.dma_start(
                out_d[0:full, :].rearrange("(t p) j -> p t j", p=128),
                out_stage[:, 0:full // 128, :])
        if NSH > full:
            nc.sync.dma_start(out_d[full:NSH, :],
                              out_stage[0:NSH - full, NT - 1, :])

    nc.compile()
    return nc


# ----------------------------------------------------------------------------
# entry point
# ----------------------------------------------------------------------------

_CACHE = {}


def kernel(x, edge_index, edge_attr, W1, att_src1, att_dst1, b1,
           W2, att_src2, att_dst2, b2, ff1_w, ff1_b, ff2_w, ff2_b):
    x = np.asarray(x, np.float32)
    edge_index = np.asarray(edge_index)
    args = [np.asarray(a, np.float32) for a in
            (W1, att_src1, att_dst1, b1, W2, att_src2, att_dst2, b2,
             ff1_w, ff1_b, ff2_w, ff2_b)]
    in_maps, sched, dims = _prep(x, edge_index, *args)
    key = (dims["N"], dims["IN"], tuple(sched.n_lo), tuple(sched.n_hi))
    if key not in _CACHE:
        _CACHE[key] = _build(sched, dims)
        _CACHE[key + ("dims",)] = dims
    nc = _CACHE[key]
    dims = _CACHE[key + ("dims",)]
    salt = np.zeros((1, 4), np.float32)
    for m in in_maps:
        m[dims["salt_name"]] = salt
    res = run_bass_kernel_spmd(nc, in_maps, list(range(NCORES))).results
    out = np.concatenate([res[k]["out"] for k in range(NCORES)], axis=0)
    return out.astype(np.float32)
